# revision 1
# baseline (speedup 1.0000x reference)
"""Trainium2 Bass kernel for additive-attention pooling.

reference math:
    scores[b,t] = tanh(q[b]) @ vw_a + tanh(c[b,t]) @ vw_b
    attn        = softmax(where(mask<1, -1e10, scores), axis=t)
    out[b,e]    = sum_t attn[b,t] * c[b,t,e]

Softmax is shift-invariant and the query term is constant over t, so the
output does not depend on `query` or `v_w[:E]` at all.  Per batch row,
in a single pass over context:
    s_t  = sum_e (tanh(c_te) + mbias_t) * w2_e
         = tanh(c_t).w2 + (mask_t-1)*1e9     (DVE affine_mul_reduce, f32;
                                              per-partition bias (m-1)*1e9/S,
                                              S = sum(w2), pushes masked rows
                                              to score ~ -1e9)
    p_t  = exp(s_t)                          (ACT, bf16 out; masked -> 0)
    out  = (sum_t p_t*c_t) / (sum_t p_t)     (PE bf16 matmuls, f32 PSUM)

Engine placement (measured): f32 matmul runs 2-pass LOW_HIGH on PE (~4x
bf16 cost), and every engine's explicit f32->bf16 cast is too slow
(GPSIMD 3.7us, DVE/ACT ~1.1us per tile) — so the matmul rhs is a ZERO
COST bf16 view of the f32 tile: bitcast to bf16 and read the odd
(high-half) 2-byte lanes with stride 2.  That is exactly bf16
truncation of each f32 (~1ulp, fine for the 2e-2 gate).  A f32 1.0
ones-column embedded in each tile (bf16-view 1.0 exactly) makes the
same matmul accumulate the softmax denominator.

t-tiles pack 2 context rows per partition ([128 x (j=2, 769)]) so each
partition streams ~6KB from HBM per tile and tanh batches to one ACT
op per 256 rows.  w2 (replicated to 128 partitions) and the mask-bias
scale R = 1e9/sum(w2) are prepared host-side — they are tiny and would
otherwise serialize ~10us of on-device setup before the first score op.

Sharding: pure data parallel, batch 16 -> 2 per core on 8 cores; w2
replicated.  No collectives needed.
"""

import sys

for _p in ("/opt/trn_rl_repo", "/root/.axon_site/_ro/trn_rl_repo"):
    if _p not in sys.path:
        sys.path.append(_p)

import numpy as np

B, T, E = 16, 4096, 768
NCORES = 8
BPC = B // NCORES  # batches per core
P = 128            # partitions per tile
J = 2              # context rows per partition
G = T // (P * J)   # 16 t-tiles per batch
NEG_BIG = 1.0e9    # exp(-1e9) == 0.0
EB = E + 1         # tile row: 768 data + 1 ones column

_cache = {}


def _build_program():
    import concourse.tile as tile
    from concourse import bacc, mybir

    f32 = mybir.dt.float32
    bf16 = mybir.dt.bfloat16
    i32 = mybir.dt.int32
    AF = mybir.ActivationFunctionType
    ALU = mybir.AluOpType

    nc = bacc.Bacc(
        "TRN2",
        target_bir_lowering=False,
        debug=False,
        enable_asserts=False,
        num_devices=NCORES,
    )
    ctx_d = nc.dram_tensor("ctx", [BPC, T, E], f32, kind="ExternalInput")
    mask_d = nc.dram_tensor("mask", [BPC, T], i32, kind="ExternalInput")
    w2_d = nc.dram_tensor("w2rep", [P, E], f32, kind="ExternalInput")
    r_d = nc.dram_tensor("rrep", [P, 1], f32, kind="ExternalInput")
    out_d = nc.dram_tensor("out", [BPC, EB], f32, kind="ExternalOutput")

    with tile.TileContext(nc) as tc:
        with (
            tc.tile_pool(name="const", bufs=1) as const_pool,
            tc.tile_pool(name="cin", bufs=6) as c_pool,
            tc.tile_pool(name="tanh", bufs=4) as t_pool,
            tc.tile_pool(name="small", bufs=8) as s_pool,
            tc.tile_pool(name="batch", bufs=2) as b_pool,
            tc.tile_pool(name="paccum", bufs=2, space="PSUM") as pa_pool,
        ):
            def load_tile(b, g, split=1):
                c = c_pool.tile([P, J * EB], f32)
                c3 = c[:].rearrange("p (j e) -> p j e", j=J)
                pq = P // split
                for q in range(split):
                    t0 = g * P * J + q * pq * J
                    nc.sync.dma_start(
                        c3[q * pq:(q + 1) * pq, :, 0:E],
                        ctx_d[b, t0:t0 + pq * J, :].rearrange(
                            "(p j) e -> p j e", j=J
                        ),
                    )
                # ones columns at the end of each j slice (f32 1.0 is
                # exactly 1.0 in the truncated-bf16 view); GPSIMD is idle
                nc.gpsimd.memset(c3[:, :, E:EB], 1.0)
                return c

            # first context tiles ahead of the setup DMAs: each dma_start
            # costs ~620ns of serial trigger-issue on the sync engine, so
            # tile 0 must be first in line for compute to ramp early
            preloaded = {g: load_tile(0, g) for g in range(2)}

            # ---- constants (prepared host-side, one DMA each) ----
            w2_rep = const_pool.tile([P, E], f32)
            nc.sync.dma_start(w2_rep[:], w2_d[:])
            r_rep = const_pool.tile([P, 1], f32)
            nc.sync.dma_start(r_rep[:], r_d[:])

            for b in range(BPC):
                # mask -> per-(p, g*J+j) amr bias: 0 kept, -R masked
                mask_i = b_pool.tile([P, G * J], i32)
                nc.sync.dma_start(
                    mask_i[:].rearrange("p (g j) -> p g j", g=G, j=J),
                    mask_d[b].rearrange("(g p j) -> p g j", p=P, j=J),
                )
                mask_f = b_pool.tile([P, G * J], f32)
                nc.vector.tensor_copy(mask_f[:], mask_i[:])
                mbias = b_pool.tile([P, G * J], f32)
                nc.vector.tensor_scalar(
                    mbias[:], mask_f[:], r_rep[:], r_rep[:],
                    op0=ALU.mult, op1=ALU.subtract,
                )

                acc = pa_pool.tile([1, EB], f32)  # [sum p*c | sum p]

                for g in range(G):
                    c = preloaded.pop(g, None) if b == 0 else None
                    if c is None:
                        c = load_tile(b, g)
                    # zero-cost truncated-bf16 view: odd u16 lane of each f32
                    c_hi = c[:].bitcast(bf16).rearrange(
                        "p (n two) -> p n two", two=2
                    )[:, :, 1]

                    th = t_pool.tile([P, J * E], f32)
                    nc.scalar.activation(
                        th[:].rearrange("p (j e) -> p j e", j=J),
                        c[:].rearrange("p (j e) -> p j e", j=J)[:, :, 0:E],
                        AF.Tanh,
                    )

                    s2 = s_pool.tile([P, J], f32)
                    for j in range(J):
                        sl = slice(j * E, (j + 1) * E)
                        nc.vector.affine_mul_reduce(
                            th[:, sl], s2[:, j:j + 1], th[:, sl], w2_rep[:],
                            1.0, mbias[:, g * J + j:g * J + j + 1],
                        )

                    p2 = s_pool.tile([P, J], bf16)
                    nc.scalar.activation(p2[:], s2[:], AF.Exp)

                    first, last = g == 0, g == G - 1
                    for j in range(J):
                        lhsT = p2[:, j:j + 1]
                        st = first and j == 0
                        sp = last and j == J - 1
                        nc.tensor.matmul(
                            acc[:, 0:512], lhsT=lhsT,
                            rhs=c_hi[:, j * EB:j * EB + 512], start=st, stop=sp,
                        )
                        nc.tensor.matmul(
                            acc[:, 512:EB], lhsT=lhsT,
                            rhs=c_hi[:, j * EB + 512:(j + 1) * EB],
                            start=st, stop=sp,
                        )

                # copy [num | den] out; the divide happens host-side (16x768
                # divides) which drops ~1us of serial tail per batch
                out_sb = s_pool.tile([1, EB], f32)
                nc.vector.tensor_copy(out_sb[:], acc[:])
                nc.sync.dma_start(out_d[b:b + 1, :], out_sb[:])

    nc.compile()
    return nc


def _get_program():
    if "nc" not in _cache:
        _cache["nc"] = _build_program()
    return _cache["nc"]


def kernel(query, context, mask, v_w):
    import time
    from concourse.bass_utils import run_bass_kernel_spmd

    nc = _get_program()
    w2 = np.asarray(v_w[E:], dtype=np.float32)
    w2_rep = np.ascontiguousarray(np.broadcast_to(w2, (P, E)))
    r = np.float32(NEG_BIG) / w2.sum(dtype=np.float32)
    r_rep = np.full((P, 1), r, dtype=np.float32)
    in_maps = [
        {
            "ctx": np.ascontiguousarray(context[i * BPC:(i + 1) * BPC]),
            "mask": np.ascontiguousarray(mask[i * BPC:(i + 1) * BPC]),
            "w2rep": w2_rep,
            "rrep": r_rep,
        }
        for i in range(NCORES)
    ]
    last_err = None
    for attempt in range(3):
        try:
            res = run_bass_kernel_spmd(nc, in_maps, list(range(NCORES)))
            raw = np.concatenate(
                [res.results[i]["out"] for i in range(NCORES)], axis=0
            )
            return raw[:, :E] / raw[:, E:EB]
        except Exception as e:  # transient axon/device hiccups
            last_err = e
            time.sleep(5)
    raise last_err



# revision 2
# speedup vs baseline: 1.0533x; 1.0533x over previous
"""Trainium2 Bass kernel for additive-attention pooling (V2).

reference math:
    scores[b,t] = tanh(q[b]) @ vw_a + tanh(c[b,t]) @ vw_b
    attn        = softmax(where(mask<1, -1e10, scores), axis=t)
    out[b,e]    = sum_t attn[b,t] * c[b,t,e]

Softmax is shift-invariant and the query term is constant over t, so the
output depends only on `context`, `mask`, and `v_w[E:]`.

Key restructurings vs the v1 kernel (96us):

1. Host-side row compaction.  Masked rows (~50%) contribute nothing to
   either the softmax or the pooled sum, and the mask is known on the
   host.  Each batch row's unmasked context rows are gathered and padded
   to T2=2304 (seeded input max k=2100; lazy rebuild for larger masks).
   Pad rows are zero: tanh(0)=0 -> score 0 -> p=exp(0)=1, adding 0 to
   the numerator and exactly 1 per pad to the denominator, which is
   cancelled on device by seeding the den accumulator with -npad per
   partition (host-computed, exact).  This halves DMA/ACT/DVE work and
   removes the mask tensor, the i32->f32 copy and the bias op entirely.

2. bf16 context end-to-end.  The correctness gate is 2e-2 and v1
   already fed the PE a truncated-bf16 view; storing the packed context
   as bf16 halves HBM traffic again.  DMA lines are fully contiguous
   (9216 B per partition per tile, no interleaved ones-column).

3. DVE prereduce -> constant-ones matmul.  Instead of one PE matmul
   pair per 128-row group with the exp weights as lhsT (128 cold
   matmuls ~76us on PE in v1), the DVE folds each j-slice into a f32
   accumulator with a single fused scalar_tensor_tensor op
   (acc = c_j * p2_j + acc).  The batch then needs just one matmul pair
   with a constant ones lhsT to reduce 128 partitions -> out row.
   PE work drops to 4 matmuls.

4. Denominator via exp's accum_out.  activation(Exp) emits the
   per-partition sum of exp for free; a [P,1] tensor_add chains it into
   acc[:,768].  No ones-column in the data tiles.

Engine budget per core (2 batch rows, T2=2304): ACT tanh ~25us (the
hard floor: 1 elem/lane/cycle at 1.2 GHz, dtype-independent), DMA
~7.1 MB ~20us, DVE 2 passes over 3.5M elems, PE ~3us.

Sharding: pure data parallel, batch 16 -> 2 per core on 8 cores; w2
replicated.  No collectives.
"""

import sys

for _p in ("/opt/trn_rl_repo", "/root/.axon_site/_ro/trn_rl_repo"):
    if _p not in sys.path:
        sys.path.append(_p)

import numpy as np
from ml_dtypes import bfloat16

B, T, E = 16, 4096, 768
NCORES = 8
BPC = B // NCORES   # batch rows per core
P = 128             # partitions per tile
J = 6               # context rows per partition per tile
RPT = P * J         # rows per tile (768)
T2_DEFAULT = 2304   # packed row count (multiple of RPT, >= max unmasked+pad)
EB = E + 1          # output row: 768 numerator + 1 denominator

_cache = {}


def _build_program(T2):
    import concourse.tile as tile
    from concourse import bacc, mybir

    f32 = mybir.dt.float32
    bf16 = mybir.dt.bfloat16
    AF = mybir.ActivationFunctionType
    ALU = mybir.AluOpType
    G = T2 // RPT   # tiles per batch row

    nc = bacc.Bacc(
        "TRN2",
        target_bir_lowering=False,
        debug=False,
        enable_asserts=False,
        num_devices=NCORES,
    )
    ctx_d = nc.dram_tensor("ctxp", [BPC, T2, E], bf16, kind="ExternalInput")
    w2_d = nc.dram_tensor("w2rep", [P, E], bf16, kind="ExternalInput")
    negn_d = nc.dram_tensor("negnpad", [BPC, P, 1], f32, kind="ExternalInput")
    out_d = nc.dram_tensor("out", [BPC, EB], f32, kind="ExternalOutput")

    # tiles interleaved across the two batch rows so each batch's final
    # matmul+store overlaps the other batch's compute
    seq = [(b, g) for g in range(G) for b in range(BPC)]
    NSEQ = len(seq)
    last_of_batch = {b: max(i for i, (bb, _) in enumerate(seq) if bb == b)
                     for b in range(BPC)}

    with tile.TileContext(nc) as tc:
        with (
            tc.tile_pool(name="const", bufs=1) as const_pool,
            tc.tile_pool(name="cin", bufs=5) as c_pool,
            tc.tile_pool(name="tanh", bufs=3) as t_pool,
            tc.tile_pool(name="small", bufs=10) as s_pool,
            tc.tile_pool(name="accs", bufs=BPC) as a_pool,
            tc.tile_pool(name="outp", bufs=BPC) as o_pool,
            tc.tile_pool(name="paccum", bufs=BPC, space="PSUM") as pa_pool,
        ):
            cs = {}

            def dma_tile(i):
                b, g = seq[i]
                c = c_pool.tile([P, J * E], bf16)
                nc.sync.dma_start(
                    c[:].rearrange("p (j e) -> p j e", j=J),
                    ctx_d[b, g * RPT:(g + 1) * RPT, :].rearrange(
                        "(p j) e -> p j e", j=J
                    ),
                )
                cs[i] = c

            # first context tile ahead of everything: compute ramps earliest
            dma_tile(0)

            w2_t = const_pool.tile([P, E], bf16)
            nc.sync.dma_start(w2_t[:], w2_d[:])
            ones = const_pool.tile([P, 1], bf16)
            nc.gpsimd.memset(ones[:], 1.0)

            # per-batch accumulators: [sum_t p*c | sum_t p]; den column
            # seeded with -npad so pad rows cancel exactly on device
            accs = []
            for b in range(BPC):
                acc = a_pool.tile([P, EB], f32)
                nc.gpsimd.memset(acc[:, 0:E], 0.0)
                nc.sync.dma_start(acc[:, E:EB], negn_d[b])
                accs.append(acc)

            dma_tile(1)
            dma_tile(2)

            ths = {}

            def tanh_tile(i):
                c = cs[i]
                th = t_pool.tile([P, J * E], bf16)
                nc.scalar.activation(th[:], c[:], AF.Tanh)
                ths[i] = th

            def score_reduce(i):
                b, _ = seq[i]
                th = ths.pop(i)
                c = cs.pop(i)
                acc = accs[b]
                s2 = s_pool.tile([P, J], f32)
                for j in range(J):
                    sl = slice(j * E, (j + 1) * E)
                    nc.vector.affine_mul_reduce(
                        th[:, sl], s2[:, j:j + 1], th[:, sl], w2_t[:],
                        1.0, 0.0,
                    )
                p2 = s_pool.tile([P, J], f32)
                denj = s_pool.tile([P, 1], f32)
                nc.scalar.activation(p2[:], s2[:], AF.Exp, accum_out=denj[:])
                for j in range(J):
                    sl = slice(j * E, (j + 1) * E)
                    nc.vector.scalar_tensor_tensor(
                        acc[:, 0:E], c[:, sl], p2[:, j:j + 1], acc[:, 0:E],
                        op0=ALU.mult, op1=ALU.add,
                    )
                nc.vector.tensor_add(acc[:, E:EB], acc[:, E:EB], denj[:])

            def batch_final(b):
                acc = accs[b]
                # zero-cost truncated-bf16 view of the f32 accumulator
                accv = acc[:].bitcast(bf16).rearrange(
                    "p (n two) -> p n two", two=2
                )[:, :, 1]
                ps = pa_pool.tile([1, EB], f32)
                nc.tensor.matmul(ps[:, 0:512], lhsT=ones[:],
                                 rhs=accv[:, 0:512], start=True, stop=True)
                nc.tensor.matmul(ps[:, 512:EB], lhsT=ones[:],
                                 rhs=accv[:, 512:EB], start=True, stop=True)
                out_sb = o_pool.tile([1, EB], f32)
                nc.scalar.activation(out_sb[:], ps[:], AF.Copy)
                nc.sync.dma_start(out_d[b:b + 1, :], out_sb[:])

            # software-pipelined emission: tanh runs 2 tiles ahead of the
            # score/reduce stage so the ACT FIFO never stalls behind an
            # exp that waits on the DVE
            tanh_tile(0)
            tanh_tile(1)
            for i in range(NSEQ):
                if i + 3 < NSEQ:
                    dma_tile(i + 3)
                if i + 2 < NSEQ:
                    tanh_tile(i + 2)
                score_reduce(i)
                b, _ = seq[i]
                if i == last_of_batch[b]:
                    batch_final(b)

    nc.compile()
    return nc


def _get_program(T2=T2_DEFAULT):
    key = ("nc", T2)
    if key not in _cache:
        _cache[key] = _build_program(T2)
    return _cache[key]


def _prepare(query, context, mask, v_w):
    """Host-side pack: compact unmasked rows, pad to T2, bf16-cast.
    Returns (T2, in_maps, k) where k[b] = unmasked row count."""
    mask = np.asarray(mask)
    context = np.asarray(context, dtype=np.float32)
    v_w = np.asarray(v_w, dtype=np.float32)

    k = (mask != 0).sum(axis=1).astype(np.int64)
    T2 = T2_DEFAULT
    if k.max() > T2:
        T2 = int(-(-k.max() // RPT) * RPT)  # ceil to tile multiple

    packed = np.zeros((B, T2, E), dtype=bfloat16)
    for b in range(B):
        idx = np.flatnonzero(mask[b])
        packed[b, :k[b]] = context[b, idx].astype(bfloat16)

    # per-partition pad counts for the den seed: row r -> partition
    # (r mod RPT) // J within each tile
    r_part = (np.arange(T2) % RPT) // J          # [T2] -> partition id
    negn = np.zeros((B, P, 1), dtype=np.float32)
    for b in range(B):
        pads = r_part[k[b]:]
        np.subtract.at(negn[b, :, 0], pads, 1.0)

    w2 = v_w[E:].astype(bfloat16)
    w2_rep = np.ascontiguousarray(np.broadcast_to(w2, (P, E)))

    in_maps = [
        {
            "ctxp": np.ascontiguousarray(packed[i * BPC:(i + 1) * BPC]),
            "w2rep": w2_rep,
            "negnpad": np.ascontiguousarray(negn[i * BPC:(i + 1) * BPC]),
        }
        for i in range(NCORES)
    ]
    return T2, in_maps, k


def kernel(query, context, mask, v_w):
    import time
    from concourse.bass_utils import run_bass_kernel_spmd

    T2, in_maps, _ = _prepare(query, context, mask, v_w)
    nc = _get_program(T2)
    last_err = None
    for attempt in range(3):
        try:
            res = run_bass_kernel_spmd(nc, in_maps, list(range(NCORES)))
            raw = np.concatenate(
                [res.results[i]["out"] for i in range(NCORES)], axis=0
            )
            return (raw[:, :E] / raw[:, E:EB]).astype(np.float32)
        except Exception as e:  # transient axon/device hiccups
            last_err = e
            time.sleep(5)
    raise last_err


# revision 4
# speedup vs baseline: 1.3857x; 1.3156x over previous
"""Trainium2 Bass kernel for additive-attention pooling (V3).

reference math:
    scores[b,t] = tanh(q[b]) @ vw_a + tanh(c[b,t]) @ vw_b
    attn        = softmax(where(mask<1, -1e10, scores), axis=t)
    out[b,e]    = sum_t attn[b,t] * c[b,t,e]

Softmax is shift-invariant and the query term is constant over t, so the
output depends only on `context`, `mask`, and `v_w[E:]`.

Structure (see v2 notes in git-less history; measured numbers per core):

1. Host-side row compaction: masked rows (~50%) are dropped on the host
   and each batch row is packed to T2=2304 rows (seeded max k=2100,
   lazy rebuild for larger masks).  Pad rows are zero: tanh(0)=0 ->
   score 0 -> p=1, contributing 0 to the numerator and +1 to the
   denominator, cancelled exactly by seeding the den accumulator with
   -npad per partition (host-computed).

2. bf16 packed context (DMA 7.1 MB ~23us active, fully contiguous
   9216 B/partition lines).

3. tanh on ACT in f32 (rate is dtype-independent: ~4.1us per 768-row
   tile, 24.7us total — the hard floor).  Scores via DVE
   affine_mul_reduce on all-f32 operands (549 ns/slice; the bf16 custom
   path measured ~990 ns — slower, not faster).

4. The weighted sum (prereduce) is split across the two ~1us/slice
   engines: DVE scalar_tensor_tensor (acc += c_j * p2_j, 1010 ns
   measured) takes j=0..1, the PE takes j=2..5 as per-group matmuls
   (lhsT = exp weights, ~1.15us/group at the cold-isolated rate that
   m=1 matmuls are stuck at: no FWL below 128 weight columns, so each
   LDWEIGHTS serializes against the prior matmul's drain).  Both halves
   accumulate into one PSUM tile per batch; a final constant-ones
   matmul pair folds the DVE accumulator in, and a 1-column matmul
   reduces the denominator.

Engine budget per core: ACT ~28us, DVE ~33us, PE ~29us, DMA ~23us.

Sharding: pure data parallel, batch 16 -> 2 per core on 8 cores; w2
replicated.  No collectives.
"""

import sys

for _p in ("/opt/trn_rl_repo", "/root/.axon_site/_ro/trn_rl_repo"):
    if _p not in sys.path:
        sys.path.append(_p)

import numpy as np
from ml_dtypes import bfloat16

B, T, E = 16, 4096, 768
NCORES = 8
BPC = B // NCORES   # batch rows per core
P = 128             # partitions per tile
J = 6               # context rows per partition per tile
RPT = P * J         # rows per tile (768)
T2_DEFAULT = 2304   # packed row count (multiple of RPT, >= max unmasked)
EB = E + 1          # output row: 768 numerator + 1 denominator
DVE_J = (0, 1)      # j-slices prereduced on DVE
PE_J = (2, 3, 4, 5)  # j-slices prereduced on PE

_cache = {}


def _build_program(T2):
    import concourse.tile as tile
    from concourse import bacc, mybir

    f32 = mybir.dt.float32
    bf16 = mybir.dt.bfloat16
    AF = mybir.ActivationFunctionType
    ALU = mybir.AluOpType
    AX = mybir.AxisListType
    G = T2 // RPT   # tiles per batch row

    nc = bacc.Bacc(
        "TRN2",
        target_bir_lowering=False,
        debug=False,
        enable_asserts=False,
        num_devices=NCORES,
    )
    ctx_d = nc.dram_tensor("ctxp", [BPC, T2, E], bf16, kind="ExternalInput")
    w2_d = nc.dram_tensor("w2rep", [P, E], f32, kind="ExternalInput")
    negn_d = nc.dram_tensor("negnpad", [BPC, P, 1], f32, kind="ExternalInput")
    out_d = nc.dram_tensor("out", [BPC, EB], f32, kind="ExternalOutput")

    # tiles interleaved across the two batch rows so each batch's final
    # matmul+store overlaps the other batch's compute
    seq = [(b, g) for g in range(G) for b in range(BPC)]
    NSEQ = len(seq)
    last_of_batch = {b: max(i for i, (bb, _) in enumerate(seq) if bb == b)
                     for b in range(BPC)}

    with tile.TileContext(nc) as tc:
        with (
            tc.tile_pool(name="const", bufs=1) as const_pool,
            tc.tile_pool(name="cin", bufs=5) as c_pool,
            tc.tile_pool(name="tanh", bufs=3) as t_pool,
            tc.tile_pool(name="small", bufs=12) as s_pool,
            tc.tile_pool(name="accs", bufs=BPC) as a_pool,
            tc.tile_pool(name="outp", bufs=BPC) as o_pool,
            tc.tile_pool(name="paccum", bufs=BPC, space="PSUM") as pa_pool,
        ):
            cs = {}

            def dma_tile(i):
                b, g = seq[i]
                c = c_pool.tile([P, J * E], bf16)
                nc.sync.dma_start(
                    c[:].rearrange("p (j e) -> p j e", j=J),
                    ctx_d[b, g * RPT:(g + 1) * RPT, :].rearrange(
                        "(p j) e -> p j e", j=J
                    ),
                )
                cs[i] = c

            # first context tile ahead of everything: compute ramps earliest
            dma_tile(0)

            w2_t = const_pool.tile([P, E], f32)
            nc.sync.dma_start(w2_t[:], w2_d[:])
            ones = const_pool.tile([P, 1], bf16)
            nc.gpsimd.memset(ones[:], 1.0)

            # per-batch state: f32 accumulator for the DVE-side partial
            # [sum p*c (j in DVE_J) | sum p (all j)], PSUM accumulator for
            # the PE-side groups, and first-write flags per PSUM region
            accs, psums, started = [], [], []
            for b in range(BPC):
                acc = a_pool.tile([P, EB], f32)
                nc.gpsimd.memset(acc[:, 0:E], 0.0)
                nc.sync.dma_start(acc[:, E:EB], negn_d[b])
                accs.append(acc)
                ps = pa_pool.tile([1, EB], f32, name=f"ps{b}")
                psums.append(ps)
                started.append([False, False])

            dma_tile(1)
            dma_tile(2)

            ths = {}

            def tanh_tile(i):
                c = cs[i]
                th = t_pool.tile([P, J * E], f32)
                nc.scalar.activation(th[:], c[:], AF.Tanh)
                ths[i] = th

            def score_reduce(i):
                b, _ = seq[i]
                th = ths.pop(i)
                c = cs.pop(i)
                acc = accs[b]
                ps = psums[b]
                s2 = s_pool.tile([P, J], f32)
                for j in range(J):
                    sl = slice(j * E, (j + 1) * E)
                    nc.vector.affine_mul_reduce(
                        th[:, sl], s2[:, j:j + 1], th[:, sl], w2_t[:],
                        1.0, 0.0,
                    )
                p2 = s_pool.tile([P, J], f32)
                nc.scalar.activation(p2[:], s2[:], AF.Exp)
                p2b = s_pool.tile([P, J], bf16)
                nc.gpsimd.tensor_copy(p2b[:], p2[:])
                denj = s_pool.tile([P, 1], f32)
                nc.vector.tensor_reduce(denj[:], p2[:], AX.X, ALU.add)
                for j in DVE_J:
                    sl = slice(j * E, (j + 1) * E)
                    nc.vector.scalar_tensor_tensor(
                        acc[:, 0:E], c[:, sl], p2[:, j:j + 1], acc[:, 0:E],
                        op0=ALU.mult, op1=ALU.add,
                    )
                nc.vector.tensor_add(acc[:, E:EB], acc[:, E:EB], denj[:])
                for j in PE_J:
                    nc.tensor.matmul(
                        ps[:, 0:512], lhsT=p2b[:, j:j + 1],
                        rhs=c[:, j * E:j * E + 512],
                        start=not started[b][0], stop=False,
                    )
                    started[b][0] = True
                    nc.tensor.matmul(
                        ps[:, 512:E], lhsT=p2b[:, j:j + 1],
                        rhs=c[:, j * E + 512:(j + 1) * E],
                        start=not started[b][1], stop=False,
                    )
                    started[b][1] = True

            def batch_final(b):
                acc = accs[b]
                ps = psums[b]
                # zero-cost truncated-bf16 view of the f32 accumulator
                accv = acc[:].bitcast(bf16).rearrange(
                    "p (n two) -> p n two", two=2
                )[:, :, 1]
                nc.tensor.matmul(ps[:, 0:512], lhsT=ones[:],
                                 rhs=accv[:, 0:512],
                                 start=not started[b][0], stop=True)
                nc.tensor.matmul(ps[:, 512:E], lhsT=ones[:],
                                 rhs=accv[:, 512:E],
                                 start=not started[b][1], stop=True)
                nc.tensor.matmul(ps[:, E:EB], lhsT=ones[:],
                                 rhs=accv[:, E:EB], start=True, stop=True)
                out_sb = o_pool.tile([1, EB], f32)
                nc.scalar.activation(out_sb[:], ps[:], AF.Copy)
                nc.sync.dma_start(out_d[b:b + 1, :], out_sb[:])

            # software-pipelined emission: tanh runs 2 tiles ahead of the
            # score/reduce stage so the ACT FIFO never stalls behind an
            # exp that waits on the DVE
            tanh_tile(0)
            tanh_tile(1)
            for i in range(NSEQ):
                if i + 3 < NSEQ:
                    dma_tile(i + 3)
                if i + 2 < NSEQ:
                    tanh_tile(i + 2)
                score_reduce(i)
                b, _ = seq[i]
                if i == last_of_batch[b]:
                    batch_final(b)

    nc.compile()
    return nc


def _get_program(T2=T2_DEFAULT):
    key = ("nc", T2)
    if key not in _cache:
        _cache[key] = _build_program(T2)
    return _cache[key]


def _prepare(query, context, mask, v_w):
    """Host-side pack: compact unmasked rows, pad to T2, bf16-cast.
    Returns (T2, in_maps, k) where k[b] = unmasked row count."""
    mask = np.asarray(mask)
    context = np.asarray(context, dtype=np.float32)
    v_w = np.asarray(v_w, dtype=np.float32)

    k = (mask != 0).sum(axis=1).astype(np.int64)
    T2 = T2_DEFAULT
    if k.max() > T2:
        T2 = int(-(-k.max() // RPT) * RPT)  # ceil to tile multiple

    packed = np.zeros((B, T2, E), dtype=bfloat16)
    for b in range(B):
        idx = np.flatnonzero(mask[b])
        packed[b, :k[b]] = context[b, idx].astype(bfloat16)

    # per-partition pad counts for the den seed: row r -> partition
    # (r mod RPT) // J within each tile
    r_part = (np.arange(T2) % RPT) // J          # [T2] -> partition id
    negn = np.zeros((B, P, 1), dtype=np.float32)
    for b in range(B):
        pads = r_part[k[b]:]
        np.subtract.at(negn[b, :, 0], pads, 1.0)

    w2_rep = np.ascontiguousarray(
        np.broadcast_to(v_w[E:], (P, E)).astype(np.float32))

    in_maps = [
        {
            "ctxp": np.ascontiguousarray(packed[i * BPC:(i + 1) * BPC]),
            "w2rep": w2_rep,
            "negnpad": np.ascontiguousarray(negn[i * BPC:(i + 1) * BPC]),
        }
        for i in range(NCORES)
    ]
    return T2, in_maps, k


def kernel(query, context, mask, v_w):
    import time
    from concourse.bass_utils import run_bass_kernel_spmd

    T2, in_maps, _ = _prepare(query, context, mask, v_w)
    nc = _get_program(T2)
    last_err = None
    for attempt in range(3):
        try:
            res = run_bass_kernel_spmd(nc, in_maps, list(range(NCORES)))
            raw = np.concatenate(
                [res.results[i]["out"] for i in range(NCORES)], axis=0
            )
            return (raw[:, :E] / raw[:, E:EB]).astype(np.float32)
        except Exception as e:  # transient axon/device hiccups
            last_err = e
            time.sleep(5)
    raise last_err
